# revision 6
# baseline (speedup 1.0000x reference)
"""Trainium2 Bass kernel for additive (Bahdanau) attention.

Reference computation (B=32, S=4096, C=U=256):
    h     = tanh(feat @ w1 + b1 + feat @ w2 + b2)      # [B,S,U]
    score = h @ wv + bv                                # [B,S,1]
    attn  = softmax(score, axis=1)                     # [B,S,1]
    ctx   = sum(attn * feat, axis=1)                   # [B,C]

Sharding: data-parallel over batch across 8 cores (4 batches/core),
weights folded host-side (W = w1+w2, b = b1+b2) and replicated.

Per-core pipeline (all matmuls bf16; PSUM accumulation fp32):
  1. SWDGE cast-DMA loads feat fp32(HBM) -> bf16(SBUF) natural layout,
     permutation s = 32p + j so each partition reads 32KB contiguous.
  2. PE transposes 128x128 tiles (is_transpose matmuls) -> PSUM bf16,
     DVE copies to SBUF -> flatT [c,s].
  3. zT = W^T @ flatT; ACT applies bias+tanh (bias is per-partition in
     the transposed orientation) -> hT bf16.
  4. score via hT-chunk-stationary matmuls (rhs = wv, N=1), which lands
     score directly in natural layout [128, 32] in PSUM.  (bv skipped:
     constant shift cancels in softmax.)  Max-subtraction skipped:
     |score| <~ 6, exp is safe in fp32.  ACT exp with accum_out gives
     per-partition partial sums.
  5. Z replicated to all partitions via ones-matmul; DVE normalizes -> attn.
  6. ctx = attn-stationary matvec accumulated over the 32 natural tiles.
"""

import numpy as np
import ml_dtypes

from concourse import bacc, mybir, tile
from concourse.bass_utils import run_bass_kernel_spmd
from concourse.masks import make_identity

B, S, C, U = 32, 4096, 256, 256
NCORES = 8
BPC = B // NCORES  # batches per core

F32 = mybir.dt.float32
BF16 = mybir.dt.bfloat16
TANH = mybir.ActivationFunctionType.Tanh
EXP = mybir.ActivationFunctionType.Exp


def build_program():
    nc = bacc.Bacc("TRN2", target_bir_lowering=False, debug=False)

    feat = nc.dram_tensor("features", [BPC, S, C], F32, kind="ExternalInput")
    w_in = nc.dram_tensor("w_comb", [C, U], BF16, kind="ExternalInput")
    wv_in = nc.dram_tensor("wv_comb", [U], BF16, kind="ExternalInput")
    b_in = nc.dram_tensor("b_comb", [U], F32, kind="ExternalInput")
    attn_out = nc.dram_tensor("attn_out", [BPC, S], F32, kind="ExternalOutput")
    ctx_out = nc.dram_tensor("ctx_out", [BPC, C], F32, kind="ExternalOutput")

    with tile.TileContext(nc) as tc:
        with (
            tc.tile_pool(name="const", bufs=1) as cpool,
            tc.tile_pool(name="flat", bufs=2) as flat_pool,
            tc.tile_pool(name="flatT", bufs=2) as flatT_pool,
            tc.tile_pool(name="hT", bufs=2) as hT_pool,
            tc.tile_pool(name="small", bufs=2) as small_pool,
            tc.tile_pool(name="pz", bufs=2, space="PSUM") as zpool,
            tc.tile_pool(name="pt", bufs=2, space="PSUM") as tpool,
            tc.tile_pool(name="pm", bufs=1, space="PSUM") as mpool,
        ):
            # ---- constants ----
            W_sb = cpool.tile([128, 2, U], BF16)   # [ck, k, u]
            wv_sb = cpool.tile([128, 2], BF16)     # [u, k]
            b_sb = cpool.tile([128, 2], F32)       # [u, m]
            id128 = cpool.tile([128, 128], BF16)
            ones128 = cpool.tile([128, 128], F32)
            nc.sync.dma_start(W_sb[:], w_in[:].rearrange("(k ck) u -> ck k u", ck=128))
            nc.sync.dma_start(wv_sb[:], wv_in[:].rearrange("(k u) -> u k", u=128))
            nc.sync.dma_start(b_sb[:], b_in[:].rearrange("(m u) -> u m", u=128))
            make_identity(nc, id128[:])
            nc.gpsimd.memset(ones128[:], 1.0)

            for b in range(BPC):
                # ---- load + cast to bf16, natural layout s = 32p + j ----
                flat_nat = flat_pool.tile([128, 32, C], BF16, tag="flat")
                nc.gpsimd.dma_start(
                    flat_nat[:], feat[b].rearrange("(p j) c -> p j c", p=128)
                )

                # ---- transpose: flatT[c-half a][c', s-col] ----
                flatT = flatT_pool.tile([128, 2, S], BF16, tag="flatT")
                for g in range(8):
                    for a in range(2):
                        pt = tpool.tile([128, 512], BF16, tag="pt")
                        for q in range(4):
                            j = 4 * g + q
                            nc.tensor.transpose(
                                pt[:, 128 * q : 128 * (q + 1)],
                                flat_nat[:, j, 128 * a : 128 * (a + 1)],
                                id128[:],
                            )
                        nc.vector.tensor_copy(
                            flatT[:, a, 512 * g : 512 * (g + 1)], pt[:]
                        )

                # ---- projection + bias + tanh: hT[u-half m][u', s-col] ----
                hT = hT_pool.tile([128, 2, S], BF16, tag="hT")
                for g2 in range(4):
                    for m in range(2):
                        pz = zpool.tile([128, 1024], F32, tag="pz")
                        for half in range(2):
                            ch = 2 * g2 + half
                            for k in range(2):
                                nc.tensor.matmul(
                                    pz[:, 512 * half : 512 * (half + 1)],
                                    W_sb[:, k, 128 * m : 128 * (m + 1)],
                                    flatT[:, k, 512 * ch : 512 * (ch + 1)],
                                    start=(k == 0),
                                    stop=(k == 1),
                                )
                        nc.scalar.activation(
                            hT[:, m, 1024 * g2 : 1024 * (g2 + 1)],
                            pz[:],
                            TANH,
                            bias=b_sb[:, m : m + 1],
                        )

                # ---- score, natural layout: pm[p', jj] = score(s = 32p' + jj)
                # hT column 128*jj + p' corresponds to s = 32p' + jj, so an
                # hT-chunk-stationary matmul against wv lands the chunk's
                # scores on partitions directly.
                pm = mpool.tile([128, 512], F32, tag="pm")
                for jj in range(32):
                    for k in range(2):
                        nc.tensor.matmul(
                            pm[:, jj : jj + 1],
                            hT[:, k, 128 * jj : 128 * (jj + 1)],
                            wv_sb[:, k : k + 1],
                            start=(k == 0),
                            stop=(k == 1),
                        )

                # ---- softmax (no max-shift; bv cancels) ----
                e_sb = small_pool.tile([128, 32], F32, tag="e")
                esum = small_pool.tile([128, 1], F32, tag="esum")
                nc.scalar.activation(e_sb[:], pm[:, 0:32], EXP, accum_out=esum[:])

                # Z replicated to all 128 partitions: ones128.T @ esum
                nc.tensor.matmul(
                    pm[:, 40:41], ones128[:], esum[:], start=True, stop=True
                )
                invZ = small_pool.tile([128, 1], F32, tag="invz")
                nc.vector.reciprocal(invZ[:], pm[:, 40:41])

                attn_bf = small_pool.tile([128, 32], BF16, tag="attnb")
                attn_f32 = small_pool.tile([128, 32], F32, tag="attnf")
                nc.vector.tensor_scalar_mul(attn_bf[:], e_sb[:], invZ[:])
                nc.vector.tensor_scalar_mul(attn_f32[:], e_sb[:], invZ[:])
                nc.sync.dma_start(
                    attn_out[b].rearrange("(p j) -> p j", p=128), attn_f32[:]
                )

                # ---- context: accumulate attn-stationary matvecs ----
                for j in range(32):
                    nc.tensor.matmul(
                        pm[0:1, 64:320],
                        attn_bf[:, j : j + 1],
                        flat_nat[:, j, :],
                        start=(j == 0),
                        stop=(j == 31),
                    )
                ctx_sb = small_pool.tile([1, C], F32, tag="ctx")
                nc.vector.tensor_copy(ctx_sb[:], pm[0:1, 64:320])
                nc.sync.dma_start(ctx_out[b], ctx_sb[:])

    nc.compile()
    return nc


_CACHED = None


def _get_program():
    global _CACHED
    if _CACHED is None:
        _CACHED = build_program()
    return _CACHED


def run(inputs, trace=False, **spmd_kwargs):
    features = np.asarray(inputs["features"], dtype=np.float32)
    w1 = np.asarray(inputs["w1"], dtype=np.float32)
    w2 = np.asarray(inputs["w2"], dtype=np.float32)
    b1 = np.asarray(inputs["b1"], dtype=np.float32)
    b2 = np.asarray(inputs["b2"], dtype=np.float32)
    wv = np.asarray(inputs["wv"], dtype=np.float32)
    # bv is a constant shift of the scores -> cancels in softmax; unused.

    w_comb = (w1 + w2).astype(ml_dtypes.bfloat16)                  # [C,U]
    wv_comb = wv.reshape(U).astype(ml_dtypes.bfloat16)             # [U]
    b_comb = (b1 + b2).astype(np.float32)                          # [U]

    nc = _get_program()
    in_maps = []
    for i in range(NCORES):
        in_maps.append(
            {
                "features": features[i * BPC : (i + 1) * BPC],
                "w_comb": w_comb,
                "wv_comb": wv_comb,
                "b_comb": b_comb,
            }
        )
    res = run_bass_kernel_spmd(
        nc, in_maps, core_ids=list(range(NCORES)), trace=trace, **spmd_kwargs
    )
    ctx = np.concatenate([r["ctx_out"] for r in res.results], axis=0)
    attn = np.concatenate([r["attn_out"] for r in res.results], axis=0)
    return ctx, attn.reshape(B, S, 1), res


def kernel(**inputs):
    ctx, attn, _ = run(inputs, trace=False)
    return ctx, attn


# revision 8
# speedup vs baseline: 65.6362x; 65.6362x over previous
"""Trainium2 Bass kernel for additive (Bahdanau) attention.

Reference computation (B=32, S=4096, C=U=256):
    h     = tanh(feat @ w1 + b1 + feat @ w2 + b2)      # [B,S,U]
    score = h @ wv + bv                                # [B,S,1]
    attn  = softmax(score, axis=1)                     # [B,S,1]
    ctx   = sum(attn * feat, axis=1)                   # [B,C]

Sharding: data-parallel over batch across 8 cores (4 batches/core),
weights folded host-side (W = w1+w2, b = b1+b2) and replicated.

Per-core pipeline (all matmuls bf16; PSUM accumulation fp32):
  1. SWDGE cast-DMA loads feat fp32(HBM) -> bf16(SBUF) natural layout,
     permutation s = 32p + j so each partition reads 32KB contiguous.
  2. PE transposes 128x128 tiles (is_transpose matmuls) -> PSUM bf16,
     DVE copies to SBUF -> flatT [c,s].
  3. zT = W^T @ flatT; ACT applies bias+tanh (bias is per-partition in
     the transposed orientation) -> hT bf16.
  4. score via hT-chunk-stationary matmuls (rhs = wv, N=1), which lands
     score directly in natural layout [128, 32] in PSUM.  (bv skipped:
     constant shift cancels in softmax.)  Max-subtraction skipped:
     |score| <~ 6, exp is safe in fp32.  ACT exp with accum_out gives
     per-partition partial sums.
  5. Z replicated to all partitions via ones-matmul; DVE normalizes -> attn.
  6. ctx = attn-stationary matvec accumulated over the 32 natural tiles.
"""

import numpy as np
import ml_dtypes

from concourse import bacc, mybir, tile
from concourse.bass_utils import run_bass_kernel_spmd
from concourse.masks import make_identity

B, S, C, U = 32, 4096, 256, 256
NCORES = 8
BPC = B // NCORES  # batches per core

F32 = mybir.dt.float32
BF16 = mybir.dt.bfloat16
TANH = mybir.ActivationFunctionType.Tanh
EXP = mybir.ActivationFunctionType.Exp


def build_program(repeats=1):
    nc = bacc.Bacc("TRN2", target_bir_lowering=False, debug=False)

    feat = nc.dram_tensor("features", [BPC, S, C], F32, kind="ExternalInput")
    w_in = nc.dram_tensor("w_comb", [C, U], BF16, kind="ExternalInput")
    wv_in = nc.dram_tensor("wv_comb", [U], BF16, kind="ExternalInput")
    b_in = nc.dram_tensor("b_comb", [U], F32, kind="ExternalInput")
    attn_out = nc.dram_tensor("attn_out", [BPC, S], F32, kind="ExternalOutput")
    ctx_out = nc.dram_tensor("ctx_out", [BPC, C], F32, kind="ExternalOutput")

    with tile.TileContext(nc) as tc:
        with (
            tc.tile_pool(name="const", bufs=1) as cpool,
            tc.tile_pool(name="flat", bufs=2) as flat_pool,
            tc.tile_pool(name="flatT", bufs=2) as flatT_pool,
            tc.tile_pool(name="hT", bufs=2) as hT_pool,
            tc.tile_pool(name="small", bufs=2) as small_pool,
            tc.tile_pool(name="pz", bufs=2, space="PSUM") as zpool,
            tc.tile_pool(name="pt", bufs=2, space="PSUM") as tpool,
            tc.tile_pool(name="pm", bufs=1, space="PSUM") as mpool,
        ):
            # ---- constants ----
            W_sb = cpool.tile([128, 2, U], BF16)   # [ck, k, u]
            wv_sb = cpool.tile([128, 2], BF16)     # [u, k]
            b_sb = cpool.tile([128, 2], F32)       # [u, m]
            id128 = cpool.tile([128, 128], BF16)
            ones128 = cpool.tile([128, 128], F32)
            nc.sync.dma_start(W_sb[:], w_in[:].rearrange("(k ck) u -> ck k u", ck=128))
            nc.sync.dma_start(wv_sb[:], wv_in[:].rearrange("(k u) -> u k", u=128))
            nc.sync.dma_start(b_sb[:], b_in[:].rearrange("(m u) -> u m", u=128))
            make_identity(nc, id128[:])
            nc.gpsimd.memset(ones128[:], 1.0)

            for b in [bb for _ in range(repeats) for bb in range(BPC)]:
                # ---- load + cast to bf16, natural layout s = 32p + j ----
                flat_nat = flat_pool.tile([128, 32, C], BF16, tag="flat")
                nc.gpsimd.dma_start(
                    flat_nat[:], feat[b].rearrange("(p j) c -> p j c", p=128)
                )

                # ---- transpose: flatT[c-half a][c', s-col] ----
                flatT = flatT_pool.tile([128, 2, S], BF16, tag="flatT")
                for g in range(8):
                    for a in range(2):
                        pt = tpool.tile([128, 512], BF16, tag="pt")
                        for q in range(4):
                            j = 4 * g + q
                            nc.tensor.transpose(
                                pt[:, 128 * q : 128 * (q + 1)],
                                flat_nat[:, j, 128 * a : 128 * (a + 1)],
                                id128[:],
                            )
                        nc.vector.tensor_copy(
                            flatT[:, a, 512 * g : 512 * (g + 1)], pt[:]
                        )

                # ---- projection + bias + tanh: hT[u-half m][u', s-col] ----
                hT = hT_pool.tile([128, 2, S], BF16, tag="hT")
                for g2 in range(4):
                    for m in range(2):
                        pz = zpool.tile([128, 1024], F32, tag="pz")
                        for half in range(2):
                            ch = 2 * g2 + half
                            for k in range(2):
                                nc.tensor.matmul(
                                    pz[:, 512 * half : 512 * (half + 1)],
                                    W_sb[:, k, 128 * m : 128 * (m + 1)],
                                    flatT[:, k, 512 * ch : 512 * (ch + 1)],
                                    start=(k == 0),
                                    stop=(k == 1),
                                )
                        nc.scalar.activation(
                            hT[:, m, 1024 * g2 : 1024 * (g2 + 1)],
                            pz[:],
                            TANH,
                            bias=b_sb[:, m : m + 1],
                        )

                # ---- score, natural layout: pm[p', jj] = score(s = 32p' + jj)
                # hT column 128*jj + p' corresponds to s = 32p' + jj, so an
                # hT-chunk-stationary matmul against wv lands the chunk's
                # scores on partitions directly.
                pm = mpool.tile([128, 512], F32, tag="pm")
                for jj in range(32):
                    for k in range(2):
                        nc.tensor.matmul(
                            pm[:, jj : jj + 1],
                            hT[:, k, 128 * jj : 128 * (jj + 1)],
                            wv_sb[:, k : k + 1],
                            start=(k == 0),
                            stop=(k == 1),
                        )

                # ---- softmax (no max-shift; bv cancels) ----
                e_sb = small_pool.tile([128, 32], F32, tag="e")
                esum = small_pool.tile([128, 1], F32, tag="esum")
                nc.scalar.activation(e_sb[:], pm[:, 0:32], EXP, accum_out=esum[:])

                # Z replicated to all 128 partitions: ones128.T @ esum
                nc.tensor.matmul(
                    pm[:, 40:41], ones128[:], esum[:], start=True, stop=True
                )
                invZ = small_pool.tile([128, 1], F32, tag="invz")
                nc.vector.reciprocal(invZ[:], pm[:, 40:41])

                attn_bf = small_pool.tile([128, 32], BF16, tag="attnb")
                attn_f32 = small_pool.tile([128, 32], F32, tag="attnf")
                nc.vector.tensor_scalar_mul(attn_bf[:], e_sb[:], invZ[:])
                nc.vector.tensor_scalar_mul(attn_f32[:], e_sb[:], invZ[:])
                nc.sync.dma_start(
                    attn_out[b].rearrange("(p j) -> p j", p=128), attn_f32[:]
                )

                # ---- context: accumulate attn-stationary matvecs ----
                for j in range(32):
                    nc.tensor.matmul(
                        pm[0:1, 64:320],
                        attn_bf[:, j : j + 1],
                        flat_nat[:, j, :],
                        start=(j == 0),
                        stop=(j == 31),
                    )
                ctx_sb = small_pool.tile([1, C], F32, tag="ctx")
                nc.vector.tensor_copy(ctx_sb[:], pm[0:1, 64:320])
                nc.sync.dma_start(ctx_out[b], ctx_sb[:])

    nc.compile()
    return nc


_CACHED = None


def _get_program():
    global _CACHED
    if _CACHED is None:
        _CACHED = build_program()
    return _CACHED


def run(inputs, trace=False, **spmd_kwargs):
    features = np.asarray(inputs["features"], dtype=np.float32)
    w1 = np.asarray(inputs["w1"], dtype=np.float32)
    w2 = np.asarray(inputs["w2"], dtype=np.float32)
    b1 = np.asarray(inputs["b1"], dtype=np.float32)
    b2 = np.asarray(inputs["b2"], dtype=np.float32)
    wv = np.asarray(inputs["wv"], dtype=np.float32)
    # bv is a constant shift of the scores -> cancels in softmax; unused.

    w_comb = (w1 + w2).astype(ml_dtypes.bfloat16)                  # [C,U]
    wv_comb = wv.reshape(U).astype(ml_dtypes.bfloat16)             # [U]
    b_comb = (b1 + b2).astype(np.float32)                          # [U]

    nc = _get_program()
    in_maps = []
    for i in range(NCORES):
        in_maps.append(
            {
                "features": features[i * BPC : (i + 1) * BPC],
                "w_comb": w_comb,
                "wv_comb": wv_comb,
                "b_comb": b_comb,
            }
        )
    res = run_bass_kernel_spmd(
        nc, in_maps, core_ids=list(range(NCORES)), trace=trace, **spmd_kwargs
    )
    ctx = np.concatenate([r["ctx_out"] for r in res.results], axis=0)
    attn = np.concatenate([r["attn_out"] for r in res.results], axis=0)
    return ctx, attn.reshape(B, S, 1), res


def kernel(**inputs):
    ctx, attn, _ = run(inputs, trace=False)
    return ctx, attn
